# revision 20
# baseline (speedup 1.0000x reference)
"""Causal multi-head attention block (B=2, S=2048, M=1024, H=16, D=64) for 8
Trainium2 NeuronCores.

Sharding: tensor-parallel over heads (2 heads per core). Each core computes
QKV for its heads from the full x (bf16), runs causal attention, then two
per-batch AllToAlls re-shard z so every core computes a 256-row slice of
each batch's output projection against the full W_proj. The batch-0
collective overlaps batch-1 attention; the batch-1 collective overlaps the
batch-0 projection. All matmuls run in bf16 (full PE rate) with fp32 PSUM
accumulation.

Self-contained: hardcodes all shapes; host-side numpy only shards/casts
inputs and concatenates outputs.
"""

import numpy as np

import concourse.bass as bass
import concourse.bacc as bacc
import concourse.mybir as mybir
import concourse.tile as tile
from concourse.bass_utils import run_bass_kernel_spmd

B, S, M, H, D = 2, 2048, 1024, 16, 64
NC = 8
R = B * S                  # 4096 rows
HPC = H // NC              # 2 heads per core
MC = HPC * D               # 128 m-columns per core
P = 128
RB = 512                   # phase-1 row block
QB = 512                   # phase-2 query block
NRB = R // RB              # 8
NQB = S // QB              # 4 query blocks per batch
NMT = M // P               # 8 m-tiles
NVT = R // P               # 32 V row tiles
RPB = S // NC              # 256 output rows per core per batch
NHB = 2                    # half-batches per batch (collective granularity)
RH = RPB // NHB            # 128 output rows per core per (batch, half)

f32 = mybir.dt.float32
bf16 = mybir.dt.bfloat16
AF = mybir.ActivationFunctionType
ALU = mybir.AluOpType

_BUILD_CACHE = {}

TUNE = {"acc_bufs": 2, "st2_bufs": 2, "ex_bufs": 4, "xp_bufs": 3,
        "wp_late": True, "b1_st2": 3, "no_coll": False, "n_coll": 2}


def build_nc(with_bias=False, for_sim=False, phases=3, repeat=1):
    key = ("nc", with_bias, for_sim, phases, repeat,
           tuple(sorted(TUNE.items())))
    if key in _BUILD_CACHE:
        return _BUILD_CACHE[key]
    nc = bacc.Bacc("TRN2", target_bir_lowering=False, debug=False,
                   num_devices=1 if for_sim else NC)

    xT = nc.dram_tensor("xT", [M, R], bf16, kind="ExternalInput").ap()
    wq = nc.dram_tensor("wq", [M, MC], bf16, kind="ExternalInput").ap()
    wk = nc.dram_tensor("wk", [M, MC], bf16, kind="ExternalInput").ap()
    wv = nc.dram_tensor("wv", [M, MC], bf16, kind="ExternalInput").ap()
    bqkv = nc.dram_tensor("bqkv", [P, 3], f32, kind="ExternalInput").ap()
    wp = nc.dram_tensor("wp", [M, M], bf16, kind="ExternalInput").ap()
    tri1 = nc.dram_tensor("tri1", [P, 128], bf16, kind="ExternalInput").ap()
    ident_d = nc.dram_tensor("ident_d", [P, P], bf16, kind="ExternalInput").ap()
    c65 = nc.dram_tensor("c65", [P, 65], bf16, kind="ExternalInput").ap()

    out = nc.dram_tensor("out", [2 * RPB, M], f32, kind="ExternalOutput").ap()

    with tile.TileContext(nc) as tc:
        with (
            tc.tile_pool(name="cb", bufs=1) as cb,        # constants / persistents
            tc.tile_pool(name="dram", bufs=1, space="DRAM") as dram,
        ):
            # ---- constants ----
            wq_sb = cb.tile([P, NMT, MC], bf16)
            wk_sb = cb.tile([P, NMT, MC], bf16)
            wv_sb = cb.tile([P, NMT, MC], bf16)
            nc.sync.dma_start(wq_sb[:], wq.rearrange("(mt p) d -> p mt d", p=P))
            nc.sync.dma_start(wk_sb[:], wk.rearrange("(mt p) d -> p mt d", p=P))
            nc.sync.dma_start(wv_sb[:], wv.rearrange("(mt p) d -> p mt d", p=P))
            bias_sb = cb.tile([P, 3], f32)
            nc.sync.dma_start(bias_sb[:], bqkv[:])
            tri1_sb = cb.tile([P, 128], bf16)
            ident = cb.tile([P, P], bf16)
            c65_sb = cb.tile([P, 65], bf16)
            nc.sync.dma_start(tri1_sb[:], tri1[:])
            nc.sync.dma_start(ident[:], ident_d[:])
            nc.sync.dma_start(c65_sb[:], c65[:])

            # ---- persistent activations ----
            QT = cb.tile([P, R], bf16)        # [2h*64, rows], q pre-scaled
            KT = cb.tile([P, R], bf16)
            VA = cb.tile([P, NVT, 65], bf16)   # [V_A | ones]
            VB = cb.tile([P, NVT, P], bf16)    # [ones | 0*63 | V_B]
            ZT = cb.tile([P, R], bf16)

            # constant columns of VA/VB written once
            nc.vector.tensor_copy(VA[:, :, 64:65],
                                  c65_sb[:, None, 0:1].to_broadcast(
                                      [P, NVT, 1]))
            nc.vector.tensor_copy(VB[:, :, 0:64],
                                  c65_sb[:, None, 0:64].to_broadcast(
                                      [P, NVT, 64]))

            # phase-3 weights
            wp_sb = cb.tile([P, NMT, M], bf16)
            if not TUNE["wp_late"]:
                nc.sync.dma_start(wp_sb[:],
                                  wp.rearrange("(mt p) n -> p mt n", p=P))

            n_coll = TUNE["n_coll"]
            CW = (B * NHB // n_coll) * RH   # cols per a2a buffer
            a2a_in = [dram.tile([M, CW], bf16, name=f"a2ai{i}")
                      for i in range(n_coll)]
            a2a_out = [dram.tile([M, CW], bf16, name=f"a2ao{i}")
                       for i in range(n_coll)]

            def copy_cast(dst, src, which):
                if with_bias:
                    nc.scalar.activation(dst, src, AF.Identity,
                                         bias=bias_sb[:, which:which + 1])
                else:
                    nc.vector.tensor_copy(dst, src)

            def flush_tr(pend, ps1, acc_bufs, tp_bufs):
                # transpose the previous row block's V into VA/VB; called
                # where PE would otherwise idle (behind the next xt DMA)
                if not pend:
                    return
                rb, vt_sb = pend.pop()
                tp = ps1.tile(
                    [P, 4, P], bf16, name="tp",
                    tag="tp" if tp_bufs else "u",
                    bufs=tp_bufs if tp_bufs else acc_bufs)
                for k in range(4):
                    nc.tensor.transpose(
                        tp[:, k, :], vt_sb[:, k * P:(k + 1) * P], ident[:])
                t0 = rb * 4
                nc.vector.tensor_copy(VA[:, t0:t0 + 4, 0:64],
                                      tp[:, :, 0:64])
                nc.vector.tensor_copy(VB[:, t0:t0 + 4, 64:128],
                                      tp[:, :, 64:128])

            def emit_ph1(rb, ps1, acc_bufs, tp_bufs, xp, vp, pend):
                r0 = rb * RB
                xt = xp.tile([P, NMT, RB], bf16, tag="xt", name="xt")
                for mt in range(NMT):
                    nc.sync.dma_start(
                        xt[:, mt, :], xT[mt * P:(mt + 1) * P, r0:r0 + RB])
                flush_tr(pend, ps1, acc_bufs, tp_bufs)
                for which, (w_sb, dst) in enumerate(
                        ((wq_sb, QT), (wk_sb, KT), (wv_sb, None))):
                    acc = ps1.tile([P, RB], f32, tag="u", name="acc",
                                   bufs=acc_bufs)
                    for mt in range(NMT):
                        nc.tensor.matmul(acc[:], w_sb[:, mt, :], xt[:, mt, :],
                                         start=(mt == 0), stop=(mt == NMT - 1))
                    if dst is not None:
                        copy_cast(dst[:, r0:r0 + RB], acc[:], which)
                    else:
                        vt_sb = vp.tile([P, RB], bf16, tag="vt", name="vt_sb")
                        copy_cast(vt_sb[:], acc[:], which)
                        pend.append((rb, vt_sb))

            def emit_ph2(b, qb, ps2, exp_pool, norm_pool, st2_bufs=None):
                gr0 = b * S + qb * QB
                zt_a = ps2.tile([65, QB], f32, tag="zt", bufs=2, name="zt_a")
                zt_b = ps2.tile([P, QB], f32, tag="zt", bufs=2, name="zt_b")
                nkj = 4 * qb + 4

                def geom(t):
                    di = t - 4 * qb
                    if di < 0:
                        return 0, QB, di
                    if di <= 1:
                        return 128 * di, QB - 128 * di, di
                    if di == 2:
                        return 256, 256, di
                    return 384, 128, di

                def scores(t):
                    col_off, w, di = geom(t)
                    st2 = ps2.tile([P, 2 * QB], f32, tag="st2",
                                   bufs=st2_bufs or TUNE["st2_bufs"],
                                   name="st2")
                    for h in range(2):
                        hp = slice(64 * h, 64 * h + 64)
                        nc.tensor.matmul(
                            st2[:, h * QB:h * QB + w],
                            KT[hp, b * S + 128 * t: b * S + 128 * t + 128],
                            QT[hp, gr0 + col_off: gr0 + col_off + w],
                            start=True, stop=True)
                    return st2

                def softmax_av(t, st2):
                    col_off, w, di = geom(t)
                    ex = exp_pool.tile([P, 2, QB], bf16, tag="ex", name="ex")
                    st2v = st2.rearrange("p (h q) -> p h q", h=2)
                    nc.scalar.activation(ex[:, :, :w], st2v[:, :, :w], AF.Exp)
                    if di >= 0:
                        nc.vector.tensor_tensor(
                            ex[:, :, 0:128], ex[:, :, 0:128],
                            tri1_sb[:, None, :].to_broadcast([P, 2, 128]),
                            ALU.mult)
                    vt_idx = 16 * b + t
                    for h, (zt_x, vx) in enumerate(((zt_a, VA), (zt_b, VB))):
                        nc.tensor.matmul(
                            zt_x[:, col_off:col_off + w], vx[:, vt_idx, :],
                            ex[:, h, :w],
                            start=(t == 0), stop=(t == nkj - 1),
                            skip_group_check=True)

                # one-stage software pipeline: scores(t+1) is emitted (and
                # runs on PE) before exp(t)/z(t), so PE never waits on ACT
                prev = None
                for t in range(nkj):
                    st2 = scores(t)
                    if prev is not None:
                        softmax_av(*prev)
                    prev = (t, st2)
                softmax_av(*prev)
                # normalize into ZT
                recip = norm_pool.tile([P, QB], f32, tag="recip", name="recip")
                nc.vector.reciprocal(recip[64:65, :], zt_a[64:65, :])
                nc.vector.reciprocal(recip[0:1, :], zt_b[0:1, :])
                # copy z out of PSUM immediately so the zt slots free early
                zsrc = norm_pool.tile([P, QB], f32, tag="zc", name="zc")
                nc.vector.tensor_copy(zsrc[0:64, :], zt_a[0:64, :])
                nc.vector.tensor_copy(zsrc[64:128, :], zt_b[64:128, :])
                rowa = norm_pool.tile([1, QB], f32, tag="rowa", name="rowa")
                nc.sync.dma_start(rowa[:], recip[64:65, :])
                bca = norm_pool.tile([64, QB], f32, tag="bca", name="bca")
                bcb = norm_pool.tile([P, QB], f32, tag="bcb", name="bcb")
                nc.gpsimd.partition_broadcast(bca[:], rowa[:], channels=64)
                nc.gpsimd.partition_broadcast(bcb[:], recip[0:1, :],
                                              channels=128)
                nc.vector.tensor_tensor(ZT[0:64, gr0:gr0 + QB],
                                        zsrc[0:64, :], bca[:], ALU.mult)
                nc.vector.tensor_tensor(ZT[64:128, gr0:gr0 + QB],
                                        zsrc[64:128, :], bcb[64:128, :],
                                        ALU.mult)
                if phases >= 3:
                    # this block's 4 row-chunks go to owner cores
                    # (qb%2)*4..+3, at the (b, half)-column of the buffer
                    hb = b * NHB + qb // 2
                    buf = hb * n_coll // (B * NHB)
                    co = (hb - buf * (B * NHB // n_coll)) * RH
                    p0 = (qb % 2) * 4 * P
                    nc.sync.dma_start(
                        a2a_in[buf][p0:p0 + 4 * P, co:co + RH].rearrange(
                            "(j p) c -> p j c", j=4),
                        ZT[:, gr0:gr0 + QB].rearrange(
                            "p (j c) -> p j c", j=4))

            def do_collective(buf):
                if for_sim or TUNE["no_coll"]:
                    nc.sync.dma_start(a2a_out[buf][:], a2a_in[buf][:])
                else:
                    nc.gpsimd.collective_compute(
                        "AllToAll", ALU.bypass,
                        replica_groups=[list(range(NC))],
                        ins=[a2a_in[buf].opt()], outs=[a2a_out[buf].opt()],
                    )

            def maybe_collective(b, qb):
                # fire buffer `buf` once its last contributing block is staged
                hb = b * NHB + qb // 2
                if qb % 2 == 1 and (hb + 1) % (B * NHB // n_coll) == 0:
                    do_collective((hb + 1) * n_coll // (B * NHB) - 1)

            def emit_ph3(buf, out_pool, ps3):
                zt_sb = out_pool.tile([P, NMT, CW], bf16, bufs=1,
                                      tag=f"zt_sb{buf}", name=f"zt_sb{buf}")
                nc.sync.dma_start(
                    zt_sb[:], a2a_out[buf].rearrange("(mt p) c -> p mt c",
                                                     p=P))
                for rt in range(CW // RH):
                    hb = buf * (B * NHB // n_coll) + rt
                    os_ = out_pool.tile([P, M], f32, tag="os", name="os_")
                    for nh in range(2):
                        acc = ps3.tile([P, 512], f32, tag="o", name="acc3")
                        for mt in range(NMT):
                            nc.tensor.matmul(
                                acc[:], zt_sb[:, mt, rt * RH:(rt + 1) * RH],
                                wp_sb[:, mt, nh * 512:(nh + 1) * 512],
                                start=(mt == 0), stop=(mt == NMT - 1))
                        nc.vector.tensor_copy(
                            os_[:, nh * 512:(nh + 1) * 512], acc[:])
                    nc.sync.dma_start(out[hb * P:(hb + 1) * P, :], os_[:])

            def emit_iter(rep):
                sfx = f"_{rep}"
                with (
                    tc.tile_pool(name="xp" + sfx,
                                 bufs=TUNE["xp_bufs"]) as xp,
                    tc.tile_pool(name="vp" + sfx, bufs=2) as vp,
                    tc.tile_pool(name="ex" + sfx,
                                 bufs=TUNE["ex_bufs"]) as exp_pool,
                    tc.tile_pool(name="np" + sfx, bufs=2) as norm_pool,
                ):
                    pend = []
                    # rb0-3 with a wide PSUM pool (closes before ps2)
                    with tc.tile_pool(name="ps1a" + sfx, bufs=1,
                                      space="PSUM") as ps1a:
                        for rb in range(4):
                            emit_ph1(rb, ps1a, 4, 2, xp, vp, pend)
                        flush_tr(pend, ps1a, 4, 2)
                    # interleaved emission: batch-0 attention + rb4-7 QKV
                    with (
                        tc.tile_pool(name="ps1b" + sfx, bufs=1,
                                     space="PSUM") as ps1b,
                        tc.tile_pool(name="ps2" + sfx, bufs=1,
                                     space="PSUM") as ps2,
                    ):
                        for rb, blk in ((4, (0, 0)), (5, (0, 1)),
                                        (6, (0, 2)), (7, (0, 3))):
                            if phases >= 2:
                                emit_ph2(*blk, ps2, exp_pool, norm_pool)
                                if phases >= 3:
                                    maybe_collective(*blk)
                            emit_ph1(rb, ps1b, TUNE["acc_bufs"], 0, xp, vp,
                                     pend)
                        flush_tr(pend, ps1b, TUNE["acc_bufs"], 0)
                        if TUNE["wp_late"] and rep == 0:
                            for mt in range(NMT):
                                nc.sync.dma_start(
                                    wp_sb[:, mt, :],
                                    wp[mt * P:(mt + 1) * P, :])
                    with tc.tile_pool(name="ps2w" + sfx, bufs=1,
                                      space="PSUM") as ps2w:
                        if phases >= 2:
                            for qb in range(NQB):
                                emit_ph2(1, qb, ps2w, exp_pool, norm_pool,
                                         st2_bufs=TUNE["b1_st2"])
                                if phases >= 3:
                                    maybe_collective(1, qb)

                # ---- phase 3: output projection (overlaps b1 collectives)
                if phases >= 3:
                    with (
                        tc.tile_pool(name="op" + sfx, bufs=2) as out_pool,
                        tc.tile_pool(name="ps3" + sfx, bufs=4,
                                     space="PSUM") as ps3,
                    ):
                        for buf in range(n_coll):
                            emit_ph3(buf, out_pool, ps3)

            for rep in range(repeat):
                emit_iter(rep)

    nc.compile()
    _BUILD_CACHE[key] = nc
    return nc


def prep_inputs(x, W_attn, b_attn, W_proj, b_proj):
    import ml_dtypes
    bf = ml_dtypes.bfloat16
    x = np.asarray(x, dtype=np.float32)
    W_attn = np.asarray(W_attn, dtype=np.float32)
    b_attn = np.asarray(b_attn, dtype=np.float32)
    W_proj = np.asarray(W_proj, dtype=np.float32)

    xT = np.ascontiguousarray(x.reshape(R, M).T).astype(bf)
    tri1 = (np.arange(128)[None, :] >= np.arange(128)[:, None]).astype(bf)
    ident = np.eye(P, dtype=bf)
    c65 = np.zeros((P, 65), dtype=bf)
    c65[:, 0] = 1.0
    scale = 1.0 / np.sqrt(D)

    in_maps = []
    for c in range(NC):
        cs = slice(MC * c, MC * (c + 1))
        bq = b_attn[0 * M:1 * M][cs] * scale
        bk = b_attn[1 * M:2 * M][cs]
        bv = b_attn[2 * M:3 * M][cs]
        in_maps.append({
            "xT": xT,
            "wq": np.ascontiguousarray(
                W_attn[:, 0 * M:1 * M][:, cs] * scale).astype(bf),
            "wk": np.ascontiguousarray(W_attn[:, 1 * M:2 * M][:, cs]).astype(bf),
            "wv": np.ascontiguousarray(W_attn[:, 2 * M:3 * M][:, cs]).astype(bf),
            "bqkv": np.ascontiguousarray(np.stack([bq, bk, bv], axis=1)),
            "wp": W_proj.astype(bf),
            "tri1": tri1, "ident_d": ident, "c65": c65,
        })
    return in_maps


def postprocess(results, b_proj):
    out = np.empty((B, S, M), dtype=np.float32)
    for c in range(NC):
        o = results[c]["out"]
        for hb in range(B * NHB):
            b, h = hb // NHB, hb % NHB
            r0 = h * (S // NHB) + c * RH
            out[b, r0:r0 + RH] = o[hb * RH:(hb + 1) * RH]
    out += np.asarray(b_proj, dtype=np.float32)[None, None, :]
    return out


def kernel(x, W_attn, b_attn, W_proj, b_proj):
    nc = build_nc(with_bias=bool(np.any(np.asarray(b_attn))))
    in_maps = prep_inputs(x, W_attn, b_attn, W_proj, b_proj)
    res = run_bass_kernel_spmd(nc, in_maps, core_ids=list(range(NC)))
    return postprocess(res.results, b_proj)


# revision 21
# speedup vs baseline: 1.0318x; 1.0318x over previous
"""Causal multi-head attention block (B=2, S=2048, M=1024, H=16, D=64) for 8
Trainium2 NeuronCores.

Sharding: tensor-parallel over heads (2 heads per core). Each core computes
QKV for its heads from the full x (bf16), runs causal attention, then two
per-batch AllToAlls re-shard z so every core computes a 256-row slice of
each batch's output projection against the full W_proj. The batch-0
collective overlaps batch-1 attention; the batch-1 collective overlaps the
batch-0 projection. All matmuls run in bf16 (full PE rate) with fp32 PSUM
accumulation.

Self-contained: hardcodes all shapes; host-side numpy only shards/casts
inputs and concatenates outputs.
"""

import numpy as np

import concourse.bass as bass
import concourse.bacc as bacc
import concourse.mybir as mybir
import concourse.tile as tile
from concourse.bass_utils import run_bass_kernel_spmd

B, S, M, H, D = 2, 2048, 1024, 16, 64
NC = 8
R = B * S                  # 4096 rows
HPC = H // NC              # 2 heads per core
MC = HPC * D               # 128 m-columns per core
P = 128
RB = 512                   # phase-1 row block
QB = 512                   # phase-2 query block
NRB = R // RB              # 8
NQB = S // QB              # 4 query blocks per batch
NMT = M // P               # 8 m-tiles
NVT = R // P               # 32 V row tiles
RPB = S // NC              # 256 output rows per core per batch
NHB = 2                    # half-batches per batch (collective granularity)
RH = RPB // NHB            # 128 output rows per core per (batch, half)

f32 = mybir.dt.float32
bf16 = mybir.dt.bfloat16
AF = mybir.ActivationFunctionType
ALU = mybir.AluOpType

_BUILD_CACHE = {}

TUNE = {"acc_bufs": 2, "st2_bufs": 2, "ex_bufs": 4, "xp_bufs": 3,
        "wp_late": True, "b1_st2": 3, "no_coll": False, "n_coll": 2}


def build_nc(with_bias=False, for_sim=False, phases=3, repeat=1):
    key = ("nc", with_bias, for_sim, phases, repeat,
           tuple(sorted(TUNE.items())))
    if key in _BUILD_CACHE:
        return _BUILD_CACHE[key]
    nc = bacc.Bacc("TRN2", target_bir_lowering=False, debug=False,
                   num_devices=1 if for_sim else NC)

    xT = nc.dram_tensor("xT", [M, R], bf16, kind="ExternalInput").ap()
    wq = nc.dram_tensor("wq", [M, MC], bf16, kind="ExternalInput").ap()
    wk = nc.dram_tensor("wk", [M, MC], bf16, kind="ExternalInput").ap()
    wv = nc.dram_tensor("wv", [M, MC], bf16, kind="ExternalInput").ap()
    bqkv = nc.dram_tensor("bqkv", [P, 3], f32, kind="ExternalInput").ap()
    wp = nc.dram_tensor("wp", [M, M], bf16, kind="ExternalInput").ap()
    tri1 = nc.dram_tensor("tri1", [P, 128], bf16, kind="ExternalInput").ap()
    ident_d = nc.dram_tensor("ident_d", [P, P], bf16, kind="ExternalInput").ap()
    c65 = nc.dram_tensor("c65", [P, 65], bf16, kind="ExternalInput").ap()

    out = nc.dram_tensor("out", [2 * RPB, M], f32, kind="ExternalOutput").ap()

    with tile.TileContext(nc) as tc:
        with (
            tc.tile_pool(name="cb", bufs=1) as cb,        # constants / persistents
            tc.tile_pool(name="dram", bufs=1, space="DRAM") as dram,
        ):
            # ---- constants ----
            wq_sb = cb.tile([P, NMT, MC], bf16)
            wk_sb = cb.tile([P, NMT, MC], bf16)
            wv_sb = cb.tile([P, NMT, MC], bf16)
            nc.sync.dma_start(wq_sb[:], wq.rearrange("(mt p) d -> p mt d", p=P))
            nc.sync.dma_start(wk_sb[:], wk.rearrange("(mt p) d -> p mt d", p=P))
            nc.sync.dma_start(wv_sb[:], wv.rearrange("(mt p) d -> p mt d", p=P))
            bias_sb = cb.tile([P, 3], f32)
            nc.sync.dma_start(bias_sb[:], bqkv[:])
            tri1_sb = cb.tile([P, 128], bf16)
            ident = cb.tile([P, P], bf16)
            c65_sb = cb.tile([P, 65], bf16)
            nc.sync.dma_start(tri1_sb[:], tri1[:])
            nc.sync.dma_start(ident[:], ident_d[:])
            nc.sync.dma_start(c65_sb[:], c65[:])

            # ---- persistent activations ----
            QT = cb.tile([P, R], bf16)        # [2h*64, rows], q pre-scaled
            KT = cb.tile([P, R], bf16)
            VA = cb.tile([P, NVT, 65], bf16)   # [V_A | ones]
            VB = cb.tile([P, NVT, P], bf16)    # [ones | 0*63 | V_B]
            ZT = cb.tile([P, R], bf16)

            # constant columns of VA/VB written once
            nc.vector.tensor_copy(VA[:, :, 64:65],
                                  c65_sb[:, None, 0:1].to_broadcast(
                                      [P, NVT, 1]))
            nc.vector.tensor_copy(VB[:, :, 0:64],
                                  c65_sb[:, None, 0:64].to_broadcast(
                                      [P, NVT, 64]))

            # phase-3 weights
            wp_sb = cb.tile([P, NMT, M], bf16)
            if not TUNE["wp_late"]:
                nc.sync.dma_start(wp_sb[:],
                                  wp.rearrange("(mt p) n -> p mt n", p=P))

            n_coll = TUNE["n_coll"]
            CW = (B * NHB // n_coll) * RH   # cols per a2a buffer
            a2a_in = [dram.tile([M, CW], bf16, name=f"a2ai{i}")
                      for i in range(n_coll)]
            a2a_out = [dram.tile([M, CW], bf16, name=f"a2ao{i}")
                       for i in range(n_coll)]

            def copy_cast(dst, src, which):
                if with_bias:
                    nc.scalar.activation(dst, src, AF.Identity,
                                         bias=bias_sb[:, which:which + 1])
                else:
                    nc.vector.tensor_copy(dst, src)

            def flush_tr(pend, ps1, acc_bufs, tp_bufs):
                # transpose the previous row block's V into VA/VB; called
                # where PE would otherwise idle (behind the next xt DMA)
                if not pend:
                    return
                rb, vt_sb = pend.pop()
                tp = ps1.tile(
                    [P, 4, P], bf16, name="tp",
                    tag="tp" if tp_bufs else "u",
                    bufs=tp_bufs if tp_bufs else acc_bufs)
                for k in range(4):
                    nc.tensor.transpose(
                        tp[:, k, :], vt_sb[:, k * P:(k + 1) * P], ident[:])
                t0 = rb * 4
                nc.vector.tensor_copy(VA[:, t0:t0 + 4, 0:64],
                                      tp[:, :, 0:64])
                nc.vector.tensor_copy(VB[:, t0:t0 + 4, 64:128],
                                      tp[:, :, 64:128])

            def emit_ph1(rb, ps1, acc_bufs, tp_bufs, xp, vp, pend):
                r0 = rb * RB
                xt = xp.tile([P, NMT, RB], bf16, tag="xt", name="xt")
                nc.sync.dma_start(
                    xt[:], xT[:, r0:r0 + RB].rearrange("(mt p) r -> p mt r",
                                                       p=P))
                flush_tr(pend, ps1, acc_bufs, tp_bufs)
                for which, (w_sb, dst) in enumerate(
                        ((wq_sb, QT), (wk_sb, KT), (wv_sb, None))):
                    acc = ps1.tile([P, RB], f32, tag="u", name="acc",
                                   bufs=acc_bufs)
                    for mt in range(NMT):
                        nc.tensor.matmul(acc[:], w_sb[:, mt, :], xt[:, mt, :],
                                         start=(mt == 0), stop=(mt == NMT - 1))
                    if dst is not None:
                        copy_cast(dst[:, r0:r0 + RB], acc[:], which)
                    else:
                        vt_sb = vp.tile([P, RB], bf16, tag="vt", name="vt_sb")
                        copy_cast(vt_sb[:], acc[:], which)
                        pend.append((rb, vt_sb))

            def emit_ph2(b, qb, ps2, exp_pool, norm_pool, st2_bufs=None):
                gr0 = b * S + qb * QB
                zt_a = ps2.tile([65, QB], f32, tag="zt", bufs=2, name="zt_a")
                zt_b = ps2.tile([P, QB], f32, tag="zt", bufs=2, name="zt_b")
                nkj = 4 * qb + 4

                def geom(t):
                    di = t - 4 * qb
                    if di < 0:
                        return 0, QB, di
                    if di <= 1:
                        return 128 * di, QB - 128 * di, di
                    if di == 2:
                        return 256, 256, di
                    return 384, 128, di

                def scores(t):
                    col_off, w, di = geom(t)
                    st2 = ps2.tile([P, 2 * QB], f32, tag="st2",
                                   bufs=st2_bufs or TUNE["st2_bufs"],
                                   name="st2")
                    for h in range(2):
                        hp = slice(64 * h, 64 * h + 64)
                        nc.tensor.matmul(
                            st2[:, h * QB:h * QB + w],
                            KT[hp, b * S + 128 * t: b * S + 128 * t + 128],
                            QT[hp, gr0 + col_off: gr0 + col_off + w],
                            start=True, stop=True)
                    return st2

                def softmax_av(t, st2):
                    col_off, w, di = geom(t)
                    ex = exp_pool.tile([P, 2, QB], bf16, tag="ex", name="ex")
                    st2v = st2.rearrange("p (h q) -> p h q", h=2)
                    nc.scalar.activation(ex[:, :, :w], st2v[:, :, :w], AF.Exp)
                    if di >= 0:
                        nc.vector.tensor_tensor(
                            ex[:, :, 0:128], ex[:, :, 0:128],
                            tri1_sb[:, None, :].to_broadcast([P, 2, 128]),
                            ALU.mult)
                    vt_idx = 16 * b + t
                    for h, (zt_x, vx) in enumerate(((zt_a, VA), (zt_b, VB))):
                        nc.tensor.matmul(
                            zt_x[:, col_off:col_off + w], vx[:, vt_idx, :],
                            ex[:, h, :w],
                            start=(t == 0), stop=(t == nkj - 1),
                            skip_group_check=True)

                # one-stage software pipeline: scores(t+1) is emitted (and
                # runs on PE) before exp(t)/z(t), so PE never waits on ACT
                prev = None
                for t in range(nkj):
                    st2 = scores(t)
                    if prev is not None:
                        softmax_av(*prev)
                    prev = (t, st2)
                softmax_av(*prev)
                # normalize into ZT
                recip = norm_pool.tile([P, QB], f32, tag="recip", name="recip")
                nc.vector.reciprocal(recip[64:65, :], zt_a[64:65, :])
                nc.vector.reciprocal(recip[0:1, :], zt_b[0:1, :])
                # copy z out of PSUM immediately so the zt slots free early
                zsrc = norm_pool.tile([P, QB], f32, tag="zc", name="zc")
                nc.vector.tensor_copy(zsrc[0:64, :], zt_a[0:64, :])
                nc.vector.tensor_copy(zsrc[64:128, :], zt_b[64:128, :])
                rowa = norm_pool.tile([1, QB], f32, tag="rowa", name="rowa")
                nc.sync.dma_start(rowa[:], recip[64:65, :])
                bca = norm_pool.tile([64, QB], f32, tag="bca", name="bca")
                bcb = norm_pool.tile([P, QB], f32, tag="bcb", name="bcb")
                nc.gpsimd.partition_broadcast(bca[:], rowa[:], channels=64)
                nc.gpsimd.partition_broadcast(bcb[:], recip[0:1, :],
                                              channels=128)
                nc.vector.tensor_tensor(ZT[0:64, gr0:gr0 + QB],
                                        zsrc[0:64, :], bca[:], ALU.mult)
                nc.vector.tensor_tensor(ZT[64:128, gr0:gr0 + QB],
                                        zsrc[64:128, :], bcb[64:128, :],
                                        ALU.mult)
                if phases >= 3:
                    # this block's 4 row-chunks go to owner cores
                    # (qb%2)*4..+3, at the (b, half)-column of the buffer
                    hb = b * NHB + qb // 2
                    buf = hb * n_coll // (B * NHB)
                    co = (hb - buf * (B * NHB // n_coll)) * RH
                    p0 = (qb % 2) * 4 * P
                    nc.sync.dma_start(
                        a2a_in[buf][p0:p0 + 4 * P, co:co + RH].rearrange(
                            "(j p) c -> p j c", j=4),
                        ZT[:, gr0:gr0 + QB].rearrange(
                            "p (j c) -> p j c", j=4))

            def do_collective(buf):
                if for_sim or TUNE["no_coll"]:
                    nc.sync.dma_start(a2a_out[buf][:], a2a_in[buf][:])
                else:
                    nc.gpsimd.collective_compute(
                        "AllToAll", ALU.bypass,
                        replica_groups=[list(range(NC))],
                        ins=[a2a_in[buf].opt()], outs=[a2a_out[buf].opt()],
                    )

            def maybe_collective(b, qb):
                # fire buffer `buf` once its last contributing block is staged
                hb = b * NHB + qb // 2
                if qb % 2 == 1 and (hb + 1) % (B * NHB // n_coll) == 0:
                    do_collective((hb + 1) * n_coll // (B * NHB) - 1)

            def emit_ph3(buf, out_pool, ps3):
                zt_sb = out_pool.tile([P, NMT, CW], bf16, bufs=1,
                                      tag=f"zt_sb{buf}", name=f"zt_sb{buf}")
                nc.sync.dma_start(
                    zt_sb[:], a2a_out[buf].rearrange("(mt p) c -> p mt c",
                                                     p=P))
                for rt in range(CW // RH):
                    hb = buf * (B * NHB // n_coll) + rt
                    os_ = out_pool.tile([P, M], f32, tag="os", name="os_")
                    for nh in range(2):
                        acc = ps3.tile([P, 512], f32, tag="o", name="acc3")
                        for mt in range(NMT):
                            nc.tensor.matmul(
                                acc[:], zt_sb[:, mt, rt * RH:(rt + 1) * RH],
                                wp_sb[:, mt, nh * 512:(nh + 1) * 512],
                                start=(mt == 0), stop=(mt == NMT - 1))
                        nc.vector.tensor_copy(
                            os_[:, nh * 512:(nh + 1) * 512], acc[:])
                    nc.sync.dma_start(out[hb * P:(hb + 1) * P, :], os_[:])

            def emit_iter(rep):
                sfx = f"_{rep}"
                with (
                    tc.tile_pool(name="xp" + sfx,
                                 bufs=TUNE["xp_bufs"]) as xp,
                    tc.tile_pool(name="vp" + sfx, bufs=2) as vp,
                    tc.tile_pool(name="ex" + sfx,
                                 bufs=TUNE["ex_bufs"]) as exp_pool,
                    tc.tile_pool(name="np" + sfx, bufs=2) as norm_pool,
                ):
                    pend = []
                    # rb0-3 with a wide PSUM pool (closes before ps2)
                    with tc.tile_pool(name="ps1a" + sfx, bufs=1,
                                      space="PSUM") as ps1a:
                        for rb in range(4):
                            emit_ph1(rb, ps1a, 4, 2, xp, vp, pend)
                        flush_tr(pend, ps1a, 4, 2)
                    # interleaved emission: batch-0 attention + rb4-7 QKV
                    with (
                        tc.tile_pool(name="ps1b" + sfx, bufs=1,
                                     space="PSUM") as ps1b,
                        tc.tile_pool(name="ps2" + sfx, bufs=1,
                                     space="PSUM") as ps2,
                    ):
                        for rb, blk in ((4, (0, 0)), (5, (0, 1)),
                                        (6, (0, 2)), (7, (0, 3))):
                            if phases >= 2:
                                emit_ph2(*blk, ps2, exp_pool, norm_pool)
                                if phases >= 3:
                                    maybe_collective(*blk)
                            emit_ph1(rb, ps1b, TUNE["acc_bufs"], 0, xp, vp,
                                     pend)
                        flush_tr(pend, ps1b, TUNE["acc_bufs"], 0)
                        if TUNE["wp_late"] and rep == 0:
                            for mt in range(NMT):
                                nc.sync.dma_start(
                                    wp_sb[:, mt, :],
                                    wp[mt * P:(mt + 1) * P, :])
                    with tc.tile_pool(name="ps2w" + sfx, bufs=1,
                                      space="PSUM") as ps2w:
                        if phases >= 2:
                            for qb in range(NQB):
                                emit_ph2(1, qb, ps2w, exp_pool, norm_pool,
                                         st2_bufs=TUNE["b1_st2"])
                                if phases >= 3:
                                    maybe_collective(1, qb)

                # ---- phase 3: output projection (overlaps b1 collectives)
                if phases >= 3:
                    with (
                        tc.tile_pool(name="op" + sfx, bufs=2) as out_pool,
                        tc.tile_pool(name="ps3" + sfx, bufs=4,
                                     space="PSUM") as ps3,
                    ):
                        for buf in range(n_coll):
                            emit_ph3(buf, out_pool, ps3)

            for rep in range(repeat):
                emit_iter(rep)

    nc.compile()
    _BUILD_CACHE[key] = nc
    return nc


def prep_inputs(x, W_attn, b_attn, W_proj, b_proj):
    import ml_dtypes
    bf = ml_dtypes.bfloat16
    x = np.asarray(x, dtype=np.float32)
    W_attn = np.asarray(W_attn, dtype=np.float32)
    b_attn = np.asarray(b_attn, dtype=np.float32)
    W_proj = np.asarray(W_proj, dtype=np.float32)

    xT = np.ascontiguousarray(x.reshape(R, M).T).astype(bf)
    tri1 = (np.arange(128)[None, :] >= np.arange(128)[:, None]).astype(bf)
    ident = np.eye(P, dtype=bf)
    c65 = np.zeros((P, 65), dtype=bf)
    c65[:, 0] = 1.0
    scale = 1.0 / np.sqrt(D)

    in_maps = []
    for c in range(NC):
        cs = slice(MC * c, MC * (c + 1))
        bq = b_attn[0 * M:1 * M][cs] * scale
        bk = b_attn[1 * M:2 * M][cs]
        bv = b_attn[2 * M:3 * M][cs]
        in_maps.append({
            "xT": xT,
            "wq": np.ascontiguousarray(
                W_attn[:, 0 * M:1 * M][:, cs] * scale).astype(bf),
            "wk": np.ascontiguousarray(W_attn[:, 1 * M:2 * M][:, cs]).astype(bf),
            "wv": np.ascontiguousarray(W_attn[:, 2 * M:3 * M][:, cs]).astype(bf),
            "bqkv": np.ascontiguousarray(np.stack([bq, bk, bv], axis=1)),
            "wp": W_proj.astype(bf),
            "tri1": tri1, "ident_d": ident, "c65": c65,
        })
    return in_maps


def postprocess(results, b_proj):
    out = np.empty((B, S, M), dtype=np.float32)
    for c in range(NC):
        o = results[c]["out"]
        for hb in range(B * NHB):
            b, h = hb // NHB, hb % NHB
            r0 = h * (S // NHB) + c * RH
            out[b, r0:r0 + RH] = o[hb * RH:(hb + 1) * RH]
    out += np.asarray(b_proj, dtype=np.float32)[None, None, :]
    return out


def kernel(x, W_attn, b_attn, W_proj, b_proj):
    nc = build_nc(with_bias=bool(np.any(np.asarray(b_attn))))
    in_maps = prep_inputs(x, W_attn, b_attn, W_proj, b_proj)
    res = run_bass_kernel_spmd(nc, in_maps, core_ids=list(range(NC)))
    return postprocess(res.results, b_proj)


# revision 26
# speedup vs baseline: 1.1089x; 1.0748x over previous
"""Causal multi-head attention block (B=2, S=2048, M=1024, H=16, D=64) for 8
Trainium2 NeuronCores.

Sharding: tensor-parallel over heads (2 heads per core). Each core computes
QKV for its heads from the full x (bf16), runs causal attention, then two
per-batch AllToAlls re-shard z so every core computes a 256-row slice of
each batch's output projection against the full W_proj. The batch-0
collective overlaps batch-1 attention; the batch-1 collective overlaps the
batch-0 projection. All matmuls run in bf16 (full PE rate) with fp32 PSUM
accumulation.

Self-contained: hardcodes all shapes; host-side numpy only shards/casts
inputs and concatenates outputs.
"""

import numpy as np

import concourse.bass as bass
import concourse.bacc as bacc
import concourse.mybir as mybir
import concourse.tile as tile
from concourse.bass_utils import run_bass_kernel_spmd

B, S, M, H, D = 2, 2048, 1024, 16, 64
NC = 8
R = B * S                  # 4096 rows
HPC = H // NC              # 2 heads per core
MC = HPC * D               # 128 m-columns per core
P = 128
RB = 512                   # phase-1 row block
QB = 512                   # phase-2 query block
NRB = R // RB              # 8
NQB = S // QB              # 4 query blocks per batch
NMT = M // P               # 8 m-tiles
NVT = R // P               # 32 V row tiles
RPB = S // NC              # 256 output rows per core per batch
NHB = 2                    # half-batches per batch (collective granularity)
RH = RPB // NHB            # 128 output rows per core per (batch, half)

f32 = mybir.dt.float32
bf16 = mybir.dt.bfloat16
AF = mybir.ActivationFunctionType
ALU = mybir.AluOpType

_BUILD_CACHE = {}

TUNE = {"acc_bufs": 2, "st2_bufs": 2, "ex_bufs": 4, "xp_bufs": 3,
        "wp_late": True, "b1_st2": 2, "no_coll": False,
        # half-batches (b0h0,b0h1,b1h0,b1h1) grouped per collective: b0 as
        # one buffer (hidden under the b1 window), b1 split so only the
        # final 0.25MB exchange is exposed
        "coll_spec": ((0, 1), (2,), (3,)),
        "ph3_in_b1": True}


def build_nc(with_bias=False, for_sim=False, phases=3, repeat=1):
    key = ("nc", with_bias, for_sim, phases, repeat,
           tuple(sorted(TUNE.items())))
    if key in _BUILD_CACHE:
        return _BUILD_CACHE[key]
    nc = bacc.Bacc("TRN2", target_bir_lowering=False, debug=False,
                   num_devices=1 if for_sim else NC)

    xT = nc.dram_tensor("xT", [M, R], bf16, kind="ExternalInput").ap()
    wq = nc.dram_tensor("wq", [M, MC], bf16, kind="ExternalInput").ap()
    wk = nc.dram_tensor("wk", [M, MC], bf16, kind="ExternalInput").ap()
    wv = nc.dram_tensor("wv", [M, MC], bf16, kind="ExternalInput").ap()
    bqkv = nc.dram_tensor("bqkv", [P, 3], f32, kind="ExternalInput").ap()
    wp = nc.dram_tensor("wp", [M, M], bf16, kind="ExternalInput").ap()
    tri1 = nc.dram_tensor("tri1", [P, 128], bf16, kind="ExternalInput").ap()
    ident_d = nc.dram_tensor("ident_d", [P, P], bf16, kind="ExternalInput").ap()
    c65 = nc.dram_tensor("c65", [P, 65], bf16, kind="ExternalInput").ap()

    out = nc.dram_tensor("out", [2 * RPB, M], f32, kind="ExternalOutput").ap()

    with tile.TileContext(nc) as tc:
        with (
            tc.tile_pool(name="cb", bufs=1) as cb,        # constants / persistents
            tc.tile_pool(name="dram", bufs=1, space="DRAM") as dram,
        ):
            # ---- constants ----
            wq_sb = cb.tile([P, NMT, MC], bf16)
            wk_sb = cb.tile([P, NMT, MC], bf16)
            wv_sb = cb.tile([P, NMT, MC], bf16)
            nc.sync.dma_start(wq_sb[:], wq.rearrange("(mt p) d -> p mt d", p=P))
            nc.sync.dma_start(wk_sb[:], wk.rearrange("(mt p) d -> p mt d", p=P))
            nc.sync.dma_start(wv_sb[:], wv.rearrange("(mt p) d -> p mt d", p=P))
            bias_sb = cb.tile([P, 3], f32)
            nc.sync.dma_start(bias_sb[:], bqkv[:])
            tri1_sb = cb.tile([P, 128], bf16)
            ident = cb.tile([P, P], bf16)
            c65_sb = cb.tile([P, 65], bf16)
            nc.sync.dma_start(tri1_sb[:], tri1[:])
            nc.sync.dma_start(ident[:], ident_d[:])
            nc.sync.dma_start(c65_sb[:], c65[:])

            # ---- persistent activations ----
            QT = cb.tile([P, R], bf16)        # [2h*64, rows], q pre-scaled
            KT = cb.tile([P, R], bf16)
            VA = cb.tile([P, NVT, 65], bf16)   # [V_A | ones]
            VB = cb.tile([P, NVT, P], bf16)    # [ones | 0*63 | V_B]
            ZT = cb.tile([P, R], bf16)

            # constant columns of VA/VB written once
            nc.vector.tensor_copy(VA[:, :, 64:65],
                                  c65_sb[:, None, 0:1].to_broadcast(
                                      [P, NVT, 1]))
            nc.vector.tensor_copy(VB[:, :, 0:64],
                                  c65_sb[:, None, 0:64].to_broadcast(
                                      [P, NVT, 64]))

            # phase-3 weights
            wp_sb = cb.tile([P, NMT, M], bf16)
            if not TUNE["wp_late"]:
                nc.sync.dma_start(wp_sb[:],
                                  wp.rearrange("(mt p) n -> p mt n", p=P))

            coll_spec = TUNE["coll_spec"]
            a2a_in = [dram.tile([M, len(g) * RH], bf16, name=f"a2ai{i}")
                      for i, g in enumerate(coll_spec)]
            a2a_out = [dram.tile([M, len(g) * RH], bf16, name=f"a2ao{i}")
                       for i, g in enumerate(coll_spec)]

            def copy_cast(dst, src, which):
                if with_bias:
                    nc.scalar.activation(dst, src, AF.Identity,
                                         bias=bias_sb[:, which:which + 1])
                else:
                    nc.vector.tensor_copy(dst, src)

            def flush_tr(pend, ps1, acc_bufs, tp_bufs):
                # transpose the previous row block's V into VA/VB; called
                # where PE would otherwise idle (behind the next xt DMA)
                if not pend:
                    return
                rb, vt_sb = pend.pop()
                tp = ps1.tile(
                    [P, 4, P], bf16, name="tp",
                    tag="tp" if tp_bufs else "u",
                    bufs=tp_bufs if tp_bufs else acc_bufs)
                for k in range(4):
                    nc.tensor.transpose(
                        tp[:, k, :], vt_sb[:, k * P:(k + 1) * P], ident[:])
                t0 = rb * 4
                nc.vector.tensor_copy(VA[:, t0:t0 + 4, 0:64],
                                      tp[:, :, 0:64])
                nc.vector.tensor_copy(VB[:, t0:t0 + 4, 64:128],
                                      tp[:, :, 64:128])

            def emit_ph1(rb, ps1, acc_bufs, tp_bufs, xp, vp, pend):
                r0 = rb * RB
                xt = xp.tile([P, NMT, RB], bf16, tag="xt", name="xt")
                nc.sync.dma_start(
                    xt[:], xT[:, r0:r0 + RB].rearrange("(mt p) r -> p mt r",
                                                       p=P))
                flush_tr(pend, ps1, acc_bufs, tp_bufs)
                for which, (w_sb, dst) in enumerate(
                        ((wq_sb, QT), (wk_sb, KT), (wv_sb, None))):
                    acc = ps1.tile([P, RB], f32, tag="u", name="acc",
                                   bufs=acc_bufs)
                    for mt in range(NMT):
                        nc.tensor.matmul(acc[:], w_sb[:, mt, :], xt[:, mt, :],
                                         start=(mt == 0), stop=(mt == NMT - 1))
                    if dst is not None:
                        copy_cast(dst[:, r0:r0 + RB], acc[:], which)
                    else:
                        vt_sb = vp.tile([P, RB], bf16, tag="vt", name="vt_sb")
                        copy_cast(vt_sb[:], acc[:], which)
                        pend.append((rb, vt_sb))

            def emit_ph2(b, qb, ps2, exp_pool, norm_pool, st2_bufs=None):
                gr0 = b * S + qb * QB
                zt_a = ps2.tile([65, QB], f32, tag="zt", bufs=2, name="zt_a")
                zt_b = ps2.tile([P, QB], f32, tag="zt", bufs=2, name="zt_b")
                nkj = 4 * qb + 4

                def geom(t):
                    di = t - 4 * qb
                    if di < 0:
                        return 0, QB, di
                    if di <= 1:
                        return 128 * di, QB - 128 * di, di
                    if di == 2:
                        return 256, 256, di
                    return 384, 128, di

                def scores(t):
                    col_off, w, di = geom(t)
                    st2 = ps2.tile([P, 2 * QB], f32, tag="st2",
                                   bufs=st2_bufs or TUNE["st2_bufs"],
                                   name="st2")
                    for h in range(2):
                        hp = slice(64 * h, 64 * h + 64)
                        nc.tensor.matmul(
                            st2[:, h * QB:h * QB + w],
                            KT[hp, b * S + 128 * t: b * S + 128 * t + 128],
                            QT[hp, gr0 + col_off: gr0 + col_off + w],
                            start=True, stop=True)
                    return st2

                def softmax_av(t, st2):
                    col_off, w, di = geom(t)
                    ex = exp_pool.tile([P, 2, QB], bf16, tag="ex", name="ex")
                    st2v = st2.rearrange("p (h q) -> p h q", h=2)
                    nc.scalar.activation(ex[:, :, :w], st2v[:, :, :w], AF.Exp)
                    if di >= 0:
                        nc.vector.tensor_tensor(
                            ex[:, :, 0:128], ex[:, :, 0:128],
                            tri1_sb[:, None, :].to_broadcast([P, 2, 128]),
                            ALU.mult)
                    vt_idx = 16 * b + t
                    for h, (zt_x, vx) in enumerate(((zt_a, VA), (zt_b, VB))):
                        nc.tensor.matmul(
                            zt_x[:, col_off:col_off + w], vx[:, vt_idx, :],
                            ex[:, h, :w],
                            start=(t == 0), stop=(t == nkj - 1),
                            skip_group_check=True)

                # one-stage software pipeline: scores(t+1) is emitted (and
                # runs on PE) before exp(t)/z(t), so PE never waits on ACT
                prev = None
                for t in range(nkj):
                    st2 = scores(t)
                    if prev is not None:
                        softmax_av(*prev)
                    prev = (t, st2)
                softmax_av(*prev)
                # normalize into ZT
                recip = norm_pool.tile([P, QB], f32, tag="recip", name="recip")
                nc.vector.reciprocal(recip[64:65, :], zt_a[64:65, :])
                nc.vector.reciprocal(recip[0:1, :], zt_b[0:1, :])
                # copy z out of PSUM immediately so the zt slots free early
                zsrc = norm_pool.tile([P, QB], f32, tag="zc", name="zc")
                nc.vector.tensor_copy(zsrc[0:64, :], zt_a[0:64, :])
                nc.vector.tensor_copy(zsrc[64:128, :], zt_b[64:128, :])
                rowa = norm_pool.tile([1, QB], f32, tag="rowa", name="rowa")
                nc.sync.dma_start(rowa[:], recip[64:65, :])
                bca = norm_pool.tile([64, QB], f32, tag="bca", name="bca")
                bcb = norm_pool.tile([P, QB], f32, tag="bcb", name="bcb")
                nc.gpsimd.partition_broadcast(bca[:], rowa[:], channels=64)
                nc.gpsimd.partition_broadcast(bcb[:], recip[0:1, :],
                                              channels=128)
                nc.vector.tensor_tensor(ZT[0:64, gr0:gr0 + QB],
                                        zsrc[0:64, :], bca[:], ALU.mult)
                nc.vector.tensor_tensor(ZT[64:128, gr0:gr0 + QB],
                                        zsrc[64:128, :], bcb[64:128, :],
                                        ALU.mult)
                if phases >= 3:
                    # this block's 4 row-chunks go to owner cores
                    # (qb%2)*4..+3, at the (b, half)-column of the buffer
                    hb = b * NHB + qb // 2
                    buf = next(i for i, g in enumerate(coll_spec) if hb in g)
                    co = coll_spec[buf].index(hb) * RH
                    p0 = (qb % 2) * 4 * P
                    nc.sync.dma_start(
                        a2a_in[buf][p0:p0 + 4 * P, co:co + RH].rearrange(
                            "(j p) c -> p j c", j=4),
                        ZT[:, gr0:gr0 + QB].rearrange(
                            "p (j c) -> p j c", j=4))

            def do_collective(buf):
                if for_sim or TUNE["no_coll"]:
                    nc.sync.dma_start(a2a_out[buf][:], a2a_in[buf][:])
                else:
                    nc.gpsimd.collective_compute(
                        "AllToAll", ALU.bypass,
                        replica_groups=[list(range(NC))],
                        ins=[a2a_in[buf].opt()], outs=[a2a_out[buf].opt()],
                    )

            def maybe_collective(b, qb):
                # fire buffer `buf` once its last contributing block is staged
                hb = b * NHB + qb // 2
                if qb % 2 == 1:
                    for i, g in enumerate(coll_spec):
                        if g[-1] == hb:
                            do_collective(i)

            def emit_ph3(buf, out_pool, ps3):
                grp = coll_spec[buf]
                zt_sb = out_pool.tile([P, NMT, len(grp) * RH], bf16, bufs=1,
                                      tag=f"zt_sb{buf}", name=f"zt_sb{buf}")
                nc.sync.dma_start(
                    zt_sb[:], a2a_out[buf].rearrange("(mt p) c -> p mt c",
                                                     p=P))
                for rt, hb in enumerate(grp):
                    os_ = out_pool.tile([P, M], f32, tag="os", name="os_")
                    for nh in range(2):
                        acc = ps3.tile([P, 512], f32, tag="o", name="acc3")
                        for mt in range(NMT):
                            nc.tensor.matmul(
                                acc[:], zt_sb[:, mt, rt * RH:(rt + 1) * RH],
                                wp_sb[:, mt, nh * 512:(nh + 1) * 512],
                                start=(mt == 0), stop=(mt == NMT - 1))
                        nc.vector.tensor_copy(
                            os_[:, nh * 512:(nh + 1) * 512], acc[:])
                    nc.sync.dma_start(out[hb * P:(hb + 1) * P, :], os_[:])

            def emit_iter(rep):
                sfx = f"_{rep}"
                with (
                    tc.tile_pool(name="xp" + sfx,
                                 bufs=TUNE["xp_bufs"]) as xp,
                    tc.tile_pool(name="vp" + sfx, bufs=2) as vp,
                    tc.tile_pool(name="ex" + sfx,
                                 bufs=TUNE["ex_bufs"]) as exp_pool,
                    tc.tile_pool(name="np" + sfx, bufs=2) as norm_pool,
                ):
                    pend = []
                    # rb0-3 with a wide PSUM pool (closes before ps2)
                    with tc.tile_pool(name="ps1a" + sfx, bufs=1,
                                      space="PSUM") as ps1a:
                        for rb in range(4):
                            emit_ph1(rb, ps1a, 4, 2, xp, vp, pend)
                        flush_tr(pend, ps1a, 4, 2)
                    # interleaved emission: batch-0 attention + rb4-7 QKV
                    with (
                        tc.tile_pool(name="ps1b" + sfx, bufs=1,
                                     space="PSUM") as ps1b,
                        tc.tile_pool(name="ps2" + sfx, bufs=1,
                                     space="PSUM") as ps2,
                    ):
                        for rb, blk in ((4, (0, 0)), (5, (0, 1)),
                                        (6, (0, 2)), (7, (0, 3))):
                            if phases >= 2:
                                emit_ph2(*blk, ps2, exp_pool, norm_pool)
                                if phases >= 3:
                                    maybe_collective(*blk)
                            emit_ph1(rb, ps1b, TUNE["acc_bufs"], 0, xp, vp,
                                     pend)
                        flush_tr(pend, ps1b, TUNE["acc_bufs"], 0)
                        if TUNE["wp_late"] and rep == 0:
                            for mt in range(NMT):
                                nc.sync.dma_start(
                                    wp_sb[:, mt, :],
                                    wp[mt * P:(mt + 1) * P, :])
                    with tc.tile_pool(name="op" + sfx, bufs=2) as out_pool:
                        first_ph3 = 0
                        with (
                            tc.tile_pool(name="ps2w" + sfx, bufs=1,
                                         space="PSUM") as ps2w,
                            tc.tile_pool(name="ps3i" + sfx, bufs=2,
                                         space="PSUM") as ps3i,
                        ):
                            if phases >= 2:
                                for qb in range(NQB):
                                    emit_ph2(1, qb, ps2w, exp_pool,
                                             norm_pool,
                                             st2_bufs=TUNE["b1_st2"])
                                    if phases >= 3:
                                        maybe_collective(1, qb)
                                    # b0 projection runs in the ACT-bound
                                    # b1 window, where PE has slack
                                    if (phases >= 3 and qb == 2
                                            and TUNE["ph3_in_b1"]):
                                        emit_ph3(0, out_pool, ps3i)
                                        first_ph3 = 1
                        # ---- phase 3 tail (overlaps b1 collectives)
                        if phases >= 3:
                            with tc.tile_pool(name="ps3" + sfx, bufs=4,
                                              space="PSUM") as ps3:
                                for buf in range(first_ph3, len(coll_spec)):
                                    emit_ph3(buf, out_pool, ps3)

            for rep in range(repeat):
                emit_iter(rep)

    nc.compile()
    _BUILD_CACHE[key] = nc
    return nc


def prep_inputs(x, W_attn, b_attn, W_proj, b_proj):
    import ml_dtypes
    bf = ml_dtypes.bfloat16
    x = np.asarray(x, dtype=np.float32)
    W_attn = np.asarray(W_attn, dtype=np.float32)
    b_attn = np.asarray(b_attn, dtype=np.float32)
    W_proj = np.asarray(W_proj, dtype=np.float32)

    xT = np.ascontiguousarray(x.reshape(R, M).T).astype(bf)
    tri1 = (np.arange(128)[None, :] >= np.arange(128)[:, None]).astype(bf)
    ident = np.eye(P, dtype=bf)
    c65 = np.zeros((P, 65), dtype=bf)
    c65[:, 0] = 1.0
    scale = 1.0 / np.sqrt(D)

    in_maps = []
    for c in range(NC):
        cs = slice(MC * c, MC * (c + 1))
        bq = b_attn[0 * M:1 * M][cs] * scale
        bk = b_attn[1 * M:2 * M][cs]
        bv = b_attn[2 * M:3 * M][cs]
        in_maps.append({
            "xT": xT,
            "wq": np.ascontiguousarray(
                W_attn[:, 0 * M:1 * M][:, cs] * scale).astype(bf),
            "wk": np.ascontiguousarray(W_attn[:, 1 * M:2 * M][:, cs]).astype(bf),
            "wv": np.ascontiguousarray(W_attn[:, 2 * M:3 * M][:, cs]).astype(bf),
            "bqkv": np.ascontiguousarray(np.stack([bq, bk, bv], axis=1)),
            "wp": W_proj.astype(bf),
            "tri1": tri1, "ident_d": ident, "c65": c65,
        })
    return in_maps


def postprocess(results, b_proj):
    out = np.empty((B, S, M), dtype=np.float32)
    for c in range(NC):
        o = results[c]["out"]
        for hb in range(B * NHB):
            b, h = hb // NHB, hb % NHB
            r0 = h * (S // NHB) + c * RH
            out[b, r0:r0 + RH] = o[hb * RH:(hb + 1) * RH]
    out += np.asarray(b_proj, dtype=np.float32)[None, None, :]
    return out


def kernel(x, W_attn, b_attn, W_proj, b_proj):
    nc = build_nc(with_bias=bool(np.any(np.asarray(b_attn))))
    in_maps = prep_inputs(x, W_attn, b_attn, W_proj, b_proj)
    res = run_bass_kernel_spmd(nc, in_maps, core_ids=list(range(NC)))
    return postprocess(res.results, b_proj)


# revision 27
# speedup vs baseline: 1.1456x; 1.0330x over previous
"""Causal multi-head attention block (B=2, S=2048, M=1024, H=16, D=64) for 8
Trainium2 NeuronCores.

Sharding: tensor-parallel over heads (2 heads per core). Each core computes
QKV for its heads from the full x (bf16), runs causal attention, then two
per-batch AllToAlls re-shard z so every core computes a 256-row slice of
each batch's output projection against the full W_proj. The batch-0
collective overlaps batch-1 attention; the batch-1 collective overlaps the
batch-0 projection. All matmuls run in bf16 (full PE rate) with fp32 PSUM
accumulation.

Self-contained: hardcodes all shapes; host-side numpy only shards/casts
inputs and concatenates outputs.
"""

import numpy as np

import concourse.bass as bass
import concourse.bacc as bacc
import concourse.mybir as mybir
import concourse.tile as tile
from concourse.bass_utils import run_bass_kernel_spmd

B, S, M, H, D = 2, 2048, 1024, 16, 64
NC = 8
R = B * S                  # 4096 rows
HPC = H // NC              # 2 heads per core
MC = HPC * D               # 128 m-columns per core
P = 128
RB = 512                   # phase-1 row block
QB = 512                   # phase-2 query block
NRB = R // RB              # 8
NQB = S // QB              # 4 query blocks per batch
NMT = M // P               # 8 m-tiles
NVT = R // P               # 32 V row tiles
RPB = S // NC              # 256 output rows per core per batch
NHB = 2                    # half-batches per batch (collective granularity)
RH = RPB // NHB            # 128 output rows per core per (batch, half)

f32 = mybir.dt.float32
bf16 = mybir.dt.bfloat16
AF = mybir.ActivationFunctionType
ALU = mybir.AluOpType

_BUILD_CACHE = {}

TUNE = {"acc_bufs": 2, "st2_bufs": 2, "ex_bufs": 4, "xp_bufs": 3,
        "wp_late": True, "b1_st2": 2, "no_coll": False,
        # half-batches (b0h0,b0h1,b1h0,b1h1) grouped per collective: first
        # buffer fires once b1's first half is staged (hides under the rest
        # of the b1 window), so only the final 0.25MB exchange is exposed
        "coll_spec": ((0, 1, 2), (3,)),
        "ph3_in_b1": False}


def build_nc(with_bias=False, for_sim=False, phases=3, repeat=1):
    key = ("nc", with_bias, for_sim, phases, repeat,
           tuple(sorted(TUNE.items())))
    if key in _BUILD_CACHE:
        return _BUILD_CACHE[key]
    nc = bacc.Bacc("TRN2", target_bir_lowering=False, debug=False,
                   num_devices=1 if for_sim else NC)

    xT = nc.dram_tensor("xT", [M, R], bf16, kind="ExternalInput").ap()
    wq = nc.dram_tensor("wq", [M, MC], bf16, kind="ExternalInput").ap()
    wk = nc.dram_tensor("wk", [M, MC], bf16, kind="ExternalInput").ap()
    wv = nc.dram_tensor("wv", [M, MC], bf16, kind="ExternalInput").ap()
    bqkv = nc.dram_tensor("bqkv", [P, 3], f32, kind="ExternalInput").ap()
    wp = nc.dram_tensor("wp", [M, M], bf16, kind="ExternalInput").ap()
    tri1 = nc.dram_tensor("tri1", [P, 128], bf16, kind="ExternalInput").ap()
    ident_d = nc.dram_tensor("ident_d", [P, P], bf16, kind="ExternalInput").ap()
    c65 = nc.dram_tensor("c65", [P, 65], bf16, kind="ExternalInput").ap()

    out = nc.dram_tensor("out", [2 * RPB, M], f32, kind="ExternalOutput").ap()

    with tile.TileContext(nc) as tc:
        with (
            tc.tile_pool(name="cb", bufs=1) as cb,        # constants / persistents
            tc.tile_pool(name="dram", bufs=1, space="DRAM") as dram,
        ):
            # ---- constants ----
            wq_sb = cb.tile([P, NMT, MC], bf16)
            wk_sb = cb.tile([P, NMT, MC], bf16)
            wv_sb = cb.tile([P, NMT, MC], bf16)
            nc.sync.dma_start(wq_sb[:], wq.rearrange("(mt p) d -> p mt d", p=P))
            nc.sync.dma_start(wk_sb[:], wk.rearrange("(mt p) d -> p mt d", p=P))
            nc.sync.dma_start(wv_sb[:], wv.rearrange("(mt p) d -> p mt d", p=P))
            bias_sb = cb.tile([P, 3], f32)
            nc.sync.dma_start(bias_sb[:], bqkv[:])
            tri1_sb = cb.tile([P, 128], bf16)
            ident = cb.tile([P, P], bf16)
            c65_sb = cb.tile([P, 65], bf16)
            nc.sync.dma_start(tri1_sb[:], tri1[:])
            nc.sync.dma_start(ident[:], ident_d[:])
            nc.sync.dma_start(c65_sb[:], c65[:])

            # ---- persistent activations ----
            QT = cb.tile([P, R], bf16)        # [2h*64, rows], q pre-scaled
            KT = cb.tile([P, R], bf16)
            VA = cb.tile([P, NVT, 65], bf16)   # [V_A | ones]
            VB = cb.tile([P, NVT, P], bf16)    # [ones | 0*63 | V_B]
            ZT = cb.tile([P, R], bf16)

            # constant columns of VA/VB written once
            nc.vector.tensor_copy(VA[:, :, 64:65],
                                  c65_sb[:, None, 0:1].to_broadcast(
                                      [P, NVT, 1]))
            nc.vector.tensor_copy(VB[:, :, 0:64],
                                  c65_sb[:, None, 0:64].to_broadcast(
                                      [P, NVT, 64]))

            # phase-3 weights
            wp_sb = cb.tile([P, NMT, M], bf16)
            if not TUNE["wp_late"]:
                nc.sync.dma_start(wp_sb[:],
                                  wp.rearrange("(mt p) n -> p mt n", p=P))

            coll_spec = TUNE["coll_spec"]
            a2a_in = [dram.tile([M, len(g) * RH], bf16, name=f"a2ai{i}")
                      for i, g in enumerate(coll_spec)]
            a2a_out = [dram.tile([M, len(g) * RH], bf16, name=f"a2ao{i}")
                       for i, g in enumerate(coll_spec)]

            def copy_cast(dst, src, which):
                if with_bias:
                    nc.scalar.activation(dst, src, AF.Identity,
                                         bias=bias_sb[:, which:which + 1])
                else:
                    nc.vector.tensor_copy(dst, src)

            def flush_tr(pend, ps1, acc_bufs, tp_bufs):
                # transpose the previous row block's V into VA/VB; called
                # where PE would otherwise idle (behind the next xt DMA)
                if not pend:
                    return
                rb, vt_sb = pend.pop()
                tp = ps1.tile(
                    [P, 4, P], bf16, name="tp",
                    tag="tp" if tp_bufs else "u",
                    bufs=tp_bufs if tp_bufs else acc_bufs)
                for k in range(4):
                    nc.tensor.transpose(
                        tp[:, k, :], vt_sb[:, k * P:(k + 1) * P], ident[:])
                t0 = rb * 4
                nc.vector.tensor_copy(VA[:, t0:t0 + 4, 0:64],
                                      tp[:, :, 0:64])
                nc.vector.tensor_copy(VB[:, t0:t0 + 4, 64:128],
                                      tp[:, :, 64:128])

            def emit_ph1(rb, ps1, acc_bufs, tp_bufs, xp, vp, pend):
                r0 = rb * RB
                xt = xp.tile([P, NMT, RB], bf16, tag="xt", name="xt")
                nc.sync.dma_start(
                    xt[:], xT[:, r0:r0 + RB].rearrange("(mt p) r -> p mt r",
                                                       p=P))
                flush_tr(pend, ps1, acc_bufs, tp_bufs)
                for which, (w_sb, dst) in enumerate(
                        ((wq_sb, QT), (wk_sb, KT), (wv_sb, None))):
                    acc = ps1.tile([P, RB], f32, tag="u", name="acc",
                                   bufs=acc_bufs)
                    for mt in range(NMT):
                        nc.tensor.matmul(acc[:], w_sb[:, mt, :], xt[:, mt, :],
                                         start=(mt == 0), stop=(mt == NMT - 1))
                    if dst is not None:
                        copy_cast(dst[:, r0:r0 + RB], acc[:], which)
                    else:
                        vt_sb = vp.tile([P, RB], bf16, tag="vt", name="vt_sb")
                        copy_cast(vt_sb[:], acc[:], which)
                        pend.append((rb, vt_sb))

            def emit_ph2(b, qb, ps2, exp_pool, norm_pool, st2_bufs=None):
                gr0 = b * S + qb * QB
                zt_a = ps2.tile([65, QB], f32, tag="zt", bufs=2, name="zt_a")
                zt_b = ps2.tile([P, QB], f32, tag="zt", bufs=2, name="zt_b")
                nkj = 4 * qb + 4

                def geom(t):
                    di = t - 4 * qb
                    if di < 0:
                        return 0, QB, di
                    if di <= 1:
                        return 128 * di, QB - 128 * di, di
                    if di == 2:
                        return 256, 256, di
                    return 384, 128, di

                def scores(t):
                    col_off, w, di = geom(t)
                    st2 = ps2.tile([P, 2 * QB], f32, tag="st2",
                                   bufs=st2_bufs or TUNE["st2_bufs"],
                                   name="st2")
                    for h in range(2):
                        hp = slice(64 * h, 64 * h + 64)
                        nc.tensor.matmul(
                            st2[:, h * QB:h * QB + w],
                            KT[hp, b * S + 128 * t: b * S + 128 * t + 128],
                            QT[hp, gr0 + col_off: gr0 + col_off + w],
                            start=True, stop=True)
                    return st2

                def softmax_av(t, st2):
                    col_off, w, di = geom(t)
                    ex = exp_pool.tile([P, 2, QB], bf16, tag="ex", name="ex")
                    st2v = st2.rearrange("p (h q) -> p h q", h=2)
                    nc.scalar.activation(ex[:, :, :w], st2v[:, :, :w], AF.Exp)
                    if di >= 0:
                        nc.vector.tensor_tensor(
                            ex[:, :, 0:128], ex[:, :, 0:128],
                            tri1_sb[:, None, :].to_broadcast([P, 2, 128]),
                            ALU.mult)
                    vt_idx = 16 * b + t
                    for h, (zt_x, vx) in enumerate(((zt_a, VA), (zt_b, VB))):
                        nc.tensor.matmul(
                            zt_x[:, col_off:col_off + w], vx[:, vt_idx, :],
                            ex[:, h, :w],
                            start=(t == 0), stop=(t == nkj - 1),
                            skip_group_check=True)

                # one-stage software pipeline: scores(t+1) is emitted (and
                # runs on PE) before exp(t)/z(t), so PE never waits on ACT
                prev = None
                for t in range(nkj):
                    st2 = scores(t)
                    if prev is not None:
                        softmax_av(*prev)
                    prev = (t, st2)
                softmax_av(*prev)
                # normalize into ZT
                recip = norm_pool.tile([P, QB], f32, tag="recip", name="recip")
                nc.vector.reciprocal(recip[64:65, :], zt_a[64:65, :])
                nc.vector.reciprocal(recip[0:1, :], zt_b[0:1, :])
                # copy z out of PSUM immediately so the zt slots free early
                zsrc = norm_pool.tile([P, QB], f32, tag="zc", name="zc")
                nc.vector.tensor_copy(zsrc[0:64, :], zt_a[0:64, :])
                nc.vector.tensor_copy(zsrc[64:128, :], zt_b[64:128, :])
                rowa = norm_pool.tile([1, QB], f32, tag="rowa", name="rowa")
                nc.sync.dma_start(rowa[:], recip[64:65, :])
                bca = norm_pool.tile([64, QB], f32, tag="bca", name="bca")
                bcb = norm_pool.tile([P, QB], f32, tag="bcb", name="bcb")
                nc.gpsimd.partition_broadcast(bca[:], rowa[:], channels=64)
                nc.gpsimd.partition_broadcast(bcb[:], recip[0:1, :],
                                              channels=128)
                nc.vector.tensor_tensor(ZT[0:64, gr0:gr0 + QB],
                                        zsrc[0:64, :], bca[:], ALU.mult)
                nc.vector.tensor_tensor(ZT[64:128, gr0:gr0 + QB],
                                        zsrc[64:128, :], bcb[64:128, :],
                                        ALU.mult)
                if phases >= 3:
                    # this block's 4 row-chunks go to owner cores
                    # (qb%2)*4..+3, at the (b, half)-column of the buffer
                    hb = b * NHB + qb // 2
                    buf = next(i for i, g in enumerate(coll_spec) if hb in g)
                    co = coll_spec[buf].index(hb) * RH
                    p0 = (qb % 2) * 4 * P
                    nc.sync.dma_start(
                        a2a_in[buf][p0:p0 + 4 * P, co:co + RH].rearrange(
                            "(j p) c -> p j c", j=4),
                        ZT[:, gr0:gr0 + QB].rearrange(
                            "p (j c) -> p j c", j=4))

            def do_collective(buf):
                if for_sim or TUNE["no_coll"]:
                    nc.sync.dma_start(a2a_out[buf][:], a2a_in[buf][:])
                else:
                    nc.gpsimd.collective_compute(
                        "AllToAll", ALU.bypass,
                        replica_groups=[list(range(NC))],
                        ins=[a2a_in[buf].opt()], outs=[a2a_out[buf].opt()],
                    )

            def maybe_collective(b, qb):
                # fire buffer `buf` once its last contributing block is staged
                hb = b * NHB + qb // 2
                if qb % 2 == 1:
                    for i, g in enumerate(coll_spec):
                        if g[-1] == hb:
                            do_collective(i)

            def emit_ph3(buf, out_pool, ps3):
                grp = coll_spec[buf]
                zt_sb = out_pool.tile([P, NMT, len(grp) * RH], bf16, bufs=1,
                                      tag=f"zt_sb{buf}", name=f"zt_sb{buf}")
                nc.sync.dma_start(
                    zt_sb[:], a2a_out[buf].rearrange("(mt p) c -> p mt c",
                                                     p=P))
                for rt, hb in enumerate(grp):
                    os_ = out_pool.tile([P, M], f32, tag="os", name="os_")
                    for nh in range(2):
                        acc = ps3.tile([P, 512], f32, tag="o", name="acc3")
                        for mt in range(NMT):
                            nc.tensor.matmul(
                                acc[:], zt_sb[:, mt, rt * RH:(rt + 1) * RH],
                                wp_sb[:, mt, nh * 512:(nh + 1) * 512],
                                start=(mt == 0), stop=(mt == NMT - 1))
                        nc.vector.tensor_copy(
                            os_[:, nh * 512:(nh + 1) * 512], acc[:])
                    nc.sync.dma_start(out[hb * P:(hb + 1) * P, :], os_[:])

            def emit_iter(rep):
                sfx = f"_{rep}"
                with (
                    tc.tile_pool(name="xp" + sfx,
                                 bufs=TUNE["xp_bufs"]) as xp,
                    tc.tile_pool(name="vp" + sfx, bufs=2) as vp,
                    tc.tile_pool(name="ex" + sfx,
                                 bufs=TUNE["ex_bufs"]) as exp_pool,
                    tc.tile_pool(name="np" + sfx, bufs=2) as norm_pool,
                ):
                    pend = []
                    # rb0-3 with a wide PSUM pool (closes before ps2)
                    with tc.tile_pool(name="ps1a" + sfx, bufs=1,
                                      space="PSUM") as ps1a:
                        for rb in range(4):
                            emit_ph1(rb, ps1a, 4, 2, xp, vp, pend)
                        flush_tr(pend, ps1a, 4, 2)
                    # interleaved emission: batch-0 attention + rb4-7 QKV
                    with (
                        tc.tile_pool(name="ps1b" + sfx, bufs=1,
                                     space="PSUM") as ps1b,
                        tc.tile_pool(name="ps2" + sfx, bufs=1,
                                     space="PSUM") as ps2,
                    ):
                        for rb, blk in ((4, (0, 0)), (5, (0, 1)),
                                        (6, (0, 2)), (7, (0, 3))):
                            if phases >= 2:
                                emit_ph2(*blk, ps2, exp_pool, norm_pool)
                                if phases >= 3:
                                    maybe_collective(*blk)
                            emit_ph1(rb, ps1b, TUNE["acc_bufs"], 0, xp, vp,
                                     pend)
                        flush_tr(pend, ps1b, TUNE["acc_bufs"], 0)
                        if TUNE["wp_late"] and rep == 0:
                            for mt in range(NMT):
                                nc.sync.dma_start(
                                    wp_sb[:, mt, :],
                                    wp[mt * P:(mt + 1) * P, :])
                    with tc.tile_pool(name="op" + sfx, bufs=2) as out_pool:
                        first_ph3 = 0
                        with (
                            tc.tile_pool(name="ps2w" + sfx, bufs=1,
                                         space="PSUM") as ps2w,
                            tc.tile_pool(name="ps3i" + sfx, bufs=2,
                                         space="PSUM") as ps3i,
                        ):
                            if phases >= 2:
                                for qb in range(NQB):
                                    emit_ph2(1, qb, ps2w, exp_pool,
                                             norm_pool,
                                             st2_bufs=TUNE["b1_st2"])
                                    if phases >= 3:
                                        maybe_collective(1, qb)
                                    # b0 projection runs in the ACT-bound
                                    # b1 window, where PE has slack
                                    if (phases >= 3 and qb == 2
                                            and TUNE["ph3_in_b1"]):
                                        emit_ph3(0, out_pool, ps3i)
                                        first_ph3 = 1
                        # ---- phase 3 tail (overlaps b1 collectives)
                        if phases >= 3:
                            with tc.tile_pool(name="ps3" + sfx, bufs=4,
                                              space="PSUM") as ps3:
                                for buf in range(first_ph3, len(coll_spec)):
                                    emit_ph3(buf, out_pool, ps3)

            for rep in range(repeat):
                emit_iter(rep)

    nc.compile()
    _BUILD_CACHE[key] = nc
    return nc


def prep_inputs(x, W_attn, b_attn, W_proj, b_proj):
    import ml_dtypes
    bf = ml_dtypes.bfloat16
    x = np.asarray(x, dtype=np.float32)
    W_attn = np.asarray(W_attn, dtype=np.float32)
    b_attn = np.asarray(b_attn, dtype=np.float32)
    W_proj = np.asarray(W_proj, dtype=np.float32)

    xT = np.ascontiguousarray(x.reshape(R, M).T).astype(bf)
    tri1 = (np.arange(128)[None, :] >= np.arange(128)[:, None]).astype(bf)
    ident = np.eye(P, dtype=bf)
    c65 = np.zeros((P, 65), dtype=bf)
    c65[:, 0] = 1.0
    scale = 1.0 / np.sqrt(D)

    in_maps = []
    for c in range(NC):
        cs = slice(MC * c, MC * (c + 1))
        bq = b_attn[0 * M:1 * M][cs] * scale
        bk = b_attn[1 * M:2 * M][cs]
        bv = b_attn[2 * M:3 * M][cs]
        in_maps.append({
            "xT": xT,
            "wq": np.ascontiguousarray(
                W_attn[:, 0 * M:1 * M][:, cs] * scale).astype(bf),
            "wk": np.ascontiguousarray(W_attn[:, 1 * M:2 * M][:, cs]).astype(bf),
            "wv": np.ascontiguousarray(W_attn[:, 2 * M:3 * M][:, cs]).astype(bf),
            "bqkv": np.ascontiguousarray(np.stack([bq, bk, bv], axis=1)),
            "wp": W_proj.astype(bf),
            "tri1": tri1, "ident_d": ident, "c65": c65,
        })
    return in_maps


def postprocess(results, b_proj):
    out = np.empty((B, S, M), dtype=np.float32)
    for c in range(NC):
        o = results[c]["out"]
        for hb in range(B * NHB):
            b, h = hb // NHB, hb % NHB
            r0 = h * (S // NHB) + c * RH
            out[b, r0:r0 + RH] = o[hb * RH:(hb + 1) * RH]
    out += np.asarray(b_proj, dtype=np.float32)[None, None, :]
    return out


def kernel(x, W_attn, b_attn, W_proj, b_proj):
    nc = build_nc(with_bias=bool(np.any(np.asarray(b_attn))))
    in_maps = prep_inputs(x, W_attn, b_attn, W_proj, b_proj)
    res = run_bass_kernel_spmd(nc, in_maps, core_ids=list(range(NC)))
    return postprocess(res.results, b_proj)
